# revision 33
# baseline (speedup 1.0000x reference)
"""Multi-head linear attention (elu+1 feature map) on 8 TRN2 NeuronCores.

Sharding: core c handles batch b = c//2, sequence half j = c%2 (2048 rows).
Each core computes q/k projections + phi + partial kv/z for its rows,
AllReduces kv/z across the (b, j) pair, then computes num/den/ctx and the
output projection for its rows. All matmuls in bf16 (fp32 PSUM accumulate).

v2 design notes (vs baseline):
  - v projection/eviction eliminated: kv = phi(k)^T (x Wv) = (phi(k)^T x) Wv.
    M = kf^T [x | 1] accumulates in PSUM per pair (z rides along as col 128);
    M is evicted, transposed on PE (host-provided identity), and one matmul
    with block-diag Wv yields kv.
  - phi(x)=elu(x)+1 computed in 3 passes (E=exp(x) fused with PSUM eviction
    on ACT; u=min(E-1,0) on Pool; r=max(x_psum,u) on DVE; +1 on DVE 4x) or
    4 passes (A=x+1 on ACT; m=min(A,1)-1 on Pool; exp on ACT; max on DVE 2x),
    per-slot choice balancing ACT vs DVE occupancy.
  - k-side is group-outer (chunks stream in with the xsm DMA) with a lag-2
    software pipeline between projection matmuls and M/z matmuls; q-side
    512-col slots are interleaved 1:1 between k-side slots to keep PE fed.
  - num -> out-proj pipelined in 1024-row blocks; y stored as bf16 on two
    DMA queues; host casts to f32.
"""
import numpy as np
import ml_dtypes

B, S, H, Dh = 4, 4096, 16, 64
E = H * Dh
N_CORES = 8
SL = S // 2          # sequence rows per core
NPAIR = H // 2       # head pairs
EPS = 1e-6

NCHUNK = SL // 128   # 128-row chunks (16)
NG = 4               # chunk groups (4 chunks each)

_CACHE = {}


def _build_program():
    import concourse.bacc as bacc
    import concourse.mybir as mybir
    import concourse.tile as tile

    bf16 = mybir.dt.bfloat16
    f32 = mybir.dt.float32
    Act = mybir.ActivationFunctionType
    Alu = mybir.AluOpType

    nc = bacc.Bacc(None, target_bir_lowering=False, num_devices=N_CORES)

    xq = nc.dram_tensor("xqT", [E, SL], bf16, kind="ExternalInput")
    xs = nc.dram_tensor("xqS", [SL, NPAIR * 129], bf16, kind="ExternalInput")
    # wpk: [wkv_bd (8x256) | wq_bd (8x128) | ident (128)] packed per partition
    wpk = nc.dram_tensor("wpk", [128, 3200], bf16, kind="ExternalInput")
    wo = nc.dram_tensor("wo", [E, E], bf16, kind="ExternalInput")
    y = nc.dram_tensor("y", [SL, E], bf16, kind="ExternalOutput")
    kv_ar = nc.dram_tensor("kv_ar", [128, NPAIR * 129], bf16)

    with tile.TileContext(nc) as tc:
        with (
            tc.tile_pool(name="persist", bufs=1) as persist,
            tc.tile_pool(name="xp", bufs=1) as xp,
            tc.tile_pool(name="kfp", bufs=3) as kfp,
            tc.tile_pool(name="tmp", bufs=2) as tmp,
            tc.tile_pool(name="qtmp", bufs=2) as qtmp,
            tc.tile_pool(name="mtl", bufs=2) as mtl,
            tc.tile_pool(name="rbcp", bufs=2) as rbcp,
            tc.tile_pool(name="outp", bufs=3) as outp,
            tc.tile_pool(name="dram", bufs=1, space="DRAM") as dram,
        ):
            # ---- input loads, ordered for startup latency ----
            # sync queue: packed weights, then xT in two-pair blocks.
            wpk_sb = persist.tile([128, 3200], bf16)
            nc.sync.dma_start(out=wpk_sb[:], in_=wpk[:, :])
            wkv_sb = wpk_sb[:, 0:2048].rearrange("k (p m) -> k p m", m=256)
            wq_sb = wpk_sb[:, 2048:3072].rearrange("k (p m) -> k p m", m=128)
            ident_sb = wpk_sb[:, 3072:3200]
            # sync carries the critical path alone (wpk + xT pairs 0-1) so
            # the first k-slot starts ~7us; xsm (first needed at slot 2)
            # follows on sync; xT pairs 2-7 ride the scalar queue.
            xq_r = xq.rearrange("(p k) s -> k p s", k=128)
            xsm = persist.tile([128, NCHUNK, NPAIR, 129], bf16)
            xs_r = xs.rearrange("(c s) (p m) -> s c p m", s=128, m=129)
            xTs = []
            for h in range(4):
                xTh = xp.tile([128, 2, SL], bf16, tag=f"xTh{h}")
                eng = nc.sync if h == 0 else nc.scalar
                eng.dma_start(out=xTh[:], in_=xq_r[:, 2 * h:2 * h + 2])
                xTs.extend([xTh[:, 0, :], xTh[:, 1, :]])
            for g in range(NG):
                nc.sync.dma_start(
                    out=xsm[:, 4 * g:4 * (g + 1)], in_=xs_r[:, 4 * g:4 * (g + 1)]
                )
            # sync queue tail: Wo (needed only at out-proj).
            wo_sb = persist.tile([128, NPAIR, E], bf16)
            nc.sync.dma_start(
                out=wo_sb[:], in_=wo.rearrange("(k p) n -> p k n", p=128)
            )
            qfT = persist.tile([128, NPAIR, SL], bf16)
            ctxT = persist.tile([128, NPAIR, SL], bf16)
            kv_in = dram.tile([128, NPAIR * 129], bf16)

            # ---- phase K: k-side slots (two 4-pair waves), then phase Q ----
            # k-slot (g,p): 4 proj matmuls -> phi -> (lag-2) M+z matmuls.
            # Collective for each wave issues right after its tails and hides
            # under the remaining k-slots / q-phase.
            kvrd = persist.tile([128, NPAIR, 129], bf16)
            zbd = persist.tile([128, NPAIR, H], bf16)
            kvbd = persist.tile([128, NPAIR, 128], bf16)
            nc.vector.memset(zbd[:], 0.0)
            nc.vector.memset(kvbd[:], 0.0)
            groups = [[0, 1], [2, 3], [4, 5], [6, 7]]
            with (
                tc.tile_pool(name="ps_kf", bufs=2, space="PSUM") as ps_kf,
                tc.tile_pool(name="ps_m", bufs=1, space="PSUM") as ps_m,
            ):
                def k_proj(g, p, mode4, a_dve=False):
                    kfps = ps_kf.tile([128, 1024], f32, tag="kfps")
                    for c8 in range(8):
                        c = g * 8 + c8
                        nc.tensor.matmul(
                            kfps[:, c8 * 128:(c8 + 1) * 128],
                            lhsT=xTs[p][:, c * 128:(c + 1) * 128],
                            rhs=wkv_sb[:, p, 0:128],
                            start=True, stop=True,
                        )
                    kf = kfp.tile([128, 8, 128], bf16, tag="kf")
                    if mode4:
                        # A=x+1 (ACT fast PSUM path or DVE, alternating to
                        # balance engines); m=min(A,1)-1 (DVE 4x);
                        # exp in-place (ACT 1x); kf=max(A,m) (DVE 2x)
                        kA = tmp.tile([128, 1024], bf16, tag="Ek")
                        if a_dve:
                            nc.vector.tensor_scalar(
                                kA[:], kfps[:], 1.0, None, Alu.add
                            )
                        else:
                            nc.scalar.activation(
                                kA[:], kfps[:], Act.Identity, bias=1.0
                            )
                        kM = tmp.tile([128, 1024], bf16, tag="uk")
                        nc.vector.tensor_scalar(
                            kM[:], kA[:], 1.0, -1.0, Alu.min, Alu.add
                        )
                        nc.scalar.activation(kM[:], kM[:], Act.Exp)
                        nc.vector.tensor_tensor(kf[:], kA[:], kM[:], Alu.max)
                    else:
                        # E=exp(x) (ACT, evicts PSUM); u=min(E-1,0) (DVE 4x);
                        # r=max(x,u) (DVE PSUM); kf=r+1 (DVE 4x)
                        Ek = tmp.tile([128, 1024], bf16, tag="Ek")
                        nc.scalar.activation(Ek[:], kfps[:], Act.Exp)
                        uk = tmp.tile([128, 1024], bf16, tag="uk")
                        nc.vector.tensor_scalar(
                            uk[:], Ek[:], -1.0, 0.0, Alu.add, Alu.min
                        )
                        nc.vector.tensor_tensor(kf[:], kfps[:], uk[:], Alu.max)
                        nc.vector.tensor_scalar(kf[:], kf[:], 1.0, None, Alu.add)
                    return kf

                def m_mm(g, p, kf, M_t):
                    # fused M+z accumulation: rhs = [x-slice | ones] (129 cols)
                    for c8 in range(8):
                        c = g * 8 + c8
                        nc.tensor.matmul(
                            M_t[:, 0:129],
                            lhsT=kf[:, c8, :],
                            rhs=xsm[:, c, p, :],
                            start=(c == 0), stop=(c == NCHUNK - 1),
                        )

                def tail(p, M_t):
                    M_sb = mtl.tile([128, 129], bf16, tag="Msb")
                    nc.vector.tensor_copy(M_sb[:], M_t[:, 0:129])
                    Mt_ps = M_t[:, 192:256].bitcast(bf16)
                    nc.tensor.transpose(Mt_ps, M_sb[:, 0:128], ident_sb)
                    Mt_sb = mtl.tile([128, 128], bf16, tag="Mtsb")
                    nc.vector.tensor_copy(Mt_sb[:], Mt_ps)
                    kvps = M_t[:, 256:384]
                    nc.tensor.matmul(
                        kvps, lhsT=Mt_sb[:], rhs=wkv_sb[:, p, 128:256],
                        start=True, stop=True,
                    )
                    kvst = outp.tile([128, 129], bf16, tag="kvst")
                    nc.vector.tensor_copy(kvst[:, 0:128], kvps)
                    nc.vector.tensor_copy(kvst[:, 128:129], M_sb[:, 128:129])
                    nc.scalar.dma_start(
                        out=kv_in[:, p * 129:(p + 1) * 129], in_=kvst[:]
                    )

                slot = 0
                for wave in range(2):
                    M_ts = [
                        ps_m.tile([128, 512], f32, tag=f"M{i}", name=f"M{i}")
                        for i in range(4)
                    ]
                    pend = []
                    for g in range(2):
                        for pi in range(4):
                            p = wave * 4 + pi
                            kf = k_proj(g, p, mode4=(slot % 8 < 5), a_dve=(slot % 2 == 1))
                            slot += 1
                            pend.append((g, p, kf))
                            if len(pend) > 2:
                                gg, pp, kk = pend.pop(0)
                                m_mm(gg, pp, kk, M_ts[pp % 4])
                    for gg, pp, kk in pend:
                        m_mm(gg, pp, kk, M_ts[pp % 4])
                    for pi in range(4):
                        tail(wave * 4 + pi, M_ts[pi])
                # AllReduce kv/z; latency hides under the q phase.
                nc.gpsimd.collective_compute(
                    "AllReduce", Alu.add, replica_groups=groups,
                    ins=[kv_in[:]], outs=[kv_ar[:]],
                )
                nc.scalar.dma_start(
                    out=kvrd[:], in_=kv_ar.rearrange("q (p c) -> q p c", c=129)
                )

            # ---- phase Q: qf slots; den accumulates per 4-pair wave so the
            # collective latency hides under the q elementwise work ----
            with (
                tc.tile_pool(name="ps_q", bufs=2, space="PSUM") as ps_q,
                tc.tile_pool(name="ps_den", bufs=1, space="PSUM") as ps_den,
            ):
                denps = ps_den.tile([16, SL], f32)

                def q_slot(j, mode4):
                    p, qq = j // 2, j % 2
                    qs = slice(qq * 1024, (qq + 1) * 1024)
                    qps = ps_q.tile([128, 1024], f32, tag="qps")
                    nc.tensor.matmul(
                        qps[:, 0:512], lhsT=wq_sb[:, p, :],
                        rhs=xTs[p][:, qq * 1024:qq * 1024 + 512],
                        start=True, stop=True,
                    )
                    nc.tensor.matmul(
                        qps[:, 512:1024], lhsT=wq_sb[:, p, :],
                        rhs=xTs[p][:, qq * 1024 + 512:(qq + 1) * 1024],
                        start=True, stop=True,
                    )
                    if mode4:
                        qA = qtmp.tile([128, 1024], bf16, tag="qA")
                        if j % 2 == 1:
                            nc.vector.tensor_scalar(
                                qA[:], qps[:], 1.0, None, Alu.add
                            )
                        else:
                            nc.scalar.activation(
                                qA[:], qps[:], Act.Identity, bias=1.0
                            )
                        qM = qtmp.tile([128, 1024], bf16, tag="qM")
                        nc.vector.tensor_scalar(
                            qM[:], qA[:], 1.0, -1.0, Alu.min, Alu.add
                        )
                        nc.scalar.activation(qM[:], qM[:], Act.Exp)
                        nc.vector.tensor_tensor(
                            qfT[:, p, qs], qA[:], qM[:], Alu.max
                        )
                    else:
                        Eq = qtmp.tile([128, 1024], bf16, tag="qA")
                        nc.scalar.activation(Eq[:], qps[:], Act.Exp)
                        uq = qtmp.tile([128, 1024], bf16, tag="qM")
                        nc.vector.tensor_scalar(
                            uq[:], Eq[:], -1.0, 0.0, Alu.add, Alu.min
                        )
                        nc.vector.tensor_tensor(
                            qfT[:, p, qs], qps[:], uq[:], Alu.max
                        )
                        nc.vector.tensor_scalar(
                            qfT[:, p, qs], qfT[:, p, qs], 1.0, None, Alu.add
                        )

                for j in range(16):
                    q_slot(j, mode4=True)
                for p in range(NPAIR):
                    nc.vector.tensor_copy(
                        zbd[0:64, p, 2 * p:2 * p + 1], kvrd[0:64, p, 128:129]
                    )
                    nc.vector.tensor_copy(
                        zbd[64:128, p, 2 * p + 1:2 * p + 2],
                        kvrd[64:128, p, 128:129],
                    )
                    nc.vector.tensor_copy(
                        kvbd[0:64, p, 0:64], kvrd[0:64, p, 0:64]
                    )
                    nc.vector.tensor_copy(
                        kvbd[64:128, p, 64:128], kvrd[64:128, p, 64:128]
                    )
                # den/recip/store pipelined per 1024-column half so the
                # first num block starts as early as possible
                den_sb = persist.tile([16, SL], f32)
                recip = persist.tile([16, SL], bf16)
                eps_sb = persist.tile([16, 1], f32)
                nc.vector.memset(eps_sb[:], EPS)
                recip_dram = dram.tile([16, SL], bf16)
                for qch in range(2):
                    hs = slice(qch * 1024, (qch + 1) * 1024)
                    for p in range(NPAIR):
                        for q2 in range(2):
                            qs = slice(
                                qch * 1024 + q2 * 512, qch * 1024 + (q2 + 1) * 512
                            )
                            nc.tensor.matmul(
                                denps[:, qs], lhsT=zbd[:, p, :],
                                rhs=qfT[:, p, qs],
                                start=(p == 0), stop=(p == NPAIR - 1),
                            )
                    nc.scalar.activation(
                        den_sb[:, hs], denps[:, hs], Act.Identity, bias=eps_sb[:]
                    )
                    eng = nc.scalar
                    eng.add_instruction(
                        mybir.InstActivation(
                            name=nc.get_next_instruction_name(),
                            func=Act.Reciprocal,
                            ins=[
                                eng.lower_ap(den_sb[:, hs]),
                                mybir.ImmediateValue(dtype=f32, value=0.0),
                                mybir.ImmediateValue(dtype=f32, value=1.0),
                                mybir.ImmediateValue(dtype=f32, value=0.0),
                            ],
                            outs=[eng.lower_ap(recip[:, hs])],
                        )
                    )
                    nc.sync.dma_start(
                        out=recip_dram[:, hs], in_=recip[:, hs]
                    )

            # ---- num + ctx + output projection, pipelined per 1024 rows ----
            with (
                tc.tile_pool(name="ps_num", bufs=2, space="PSUM") as ps_num,
                tc.tile_pool(name="ps_o", bufs=2, space="PSUM") as ps_o,
            ):
                for qc in range(2):
                    qs = slice(qc * 1024, (qc + 1) * 1024)
                    for p in range(NPAIR):
                        rbc = rbcp.tile([128, 1024], bf16, tag="rbc")
                        eng = nc.sync if p % 2 == 0 else nc.scalar
                        eng.dma_start(
                            out=rbc[0:64, :],
                            in_=recip_dram[2 * p:2 * p + 1, qs].to_broadcast(
                                [64, 1024]
                            ),
                        )
                        eng.dma_start(
                            out=rbc[64:128, :],
                            in_=recip_dram[2 * p + 1:2 * p + 2, qs].to_broadcast(
                                [64, 1024]
                            ),
                        )
                        nps = ps_num.tile([128, 1024], f32, tag="nps")
                        nc.tensor.matmul(
                            nps[:, 0:512], lhsT=kvbd[:, p, :],
                            rhs=qfT[:, p, qc * 1024:qc * 1024 + 512],
                            start=True, stop=True,
                        )
                        nc.tensor.matmul(
                            nps[:, 512:1024], lhsT=kvbd[:, p, :],
                            rhs=qfT[:, p, qc * 1024 + 512:(qc + 1) * 1024],
                            start=True, stop=True,
                        )
                        nc.vector.tensor_tensor(
                            ctxT[:, p, qs], nps[:], rbc[:], Alu.mult
                        )
                    for s8 in range(8):
                        si = qc * 8 + s8
                        ss = slice(si * 128, (si + 1) * 128)
                        ops = ps_o.tile([128, E], f32, tag="ops")
                        for k in range(NPAIR):
                            nc.tensor.matmul(
                                ops[:, 0:512], lhsT=ctxT[:, k, ss],
                                rhs=wo_sb[:, k, 0:512],
                                start=(k == 0), stop=(k == NPAIR - 1),
                            )
                            nc.tensor.matmul(
                                ops[:, 512:E], lhsT=ctxT[:, k, ss],
                                rhs=wo_sb[:, k, 512:E],
                                start=(k == 0), stop=(k == NPAIR - 1),
                            )
                        ysb = outp.tile([128, E], bf16, tag="ysb")
                        nc.scalar.activation(ysb[:], ops[:], Act.Identity)
                        eng = nc.sync if s8 % 2 == 0 else nc.scalar
                        eng.dma_start(out=y[ss, :], in_=ysb[:])

    nc.compile()
    return nc


def _get_program():
    if "nc" not in _CACHE:
        _CACHE["nc"] = _build_program()
    return _CACHE["nc"]


def _host_prep(query, Wq, Wk, Wv, Wo):
    bf16 = ml_dtypes.bfloat16
    q_bf = np.ascontiguousarray(np.asarray(query).astype(bf16))
    wq_bd = np.zeros((NPAIR, 128, 128), dtype=bf16)
    wkv_bd = np.zeros((NPAIR, 128, 256), dtype=bf16)
    for p in range(NPAIR):
        wq_bd[p, 0:64, 0:64] = Wq[2 * p]
        wq_bd[p, 64:128, 64:128] = Wq[2 * p + 1]
        wkv_bd[p, 0:64, 0:64] = Wk[2 * p]
        wkv_bd[p, 64:128, 64:128] = Wk[2 * p + 1]
        wkv_bd[p, 0:64, 128:192] = Wv[2 * p]
        wkv_bd[p, 64:128, 192:256] = Wv[2 * p + 1]
    wo_bf = np.ascontiguousarray(np.asarray(Wo).astype(bf16))
    wpk = np.ascontiguousarray(np.concatenate([
        wkv_bd.transpose(1, 0, 2).reshape(128, NPAIR * 256),
        wq_bd.transpose(1, 0, 2).reshape(128, NPAIR * 128),
        np.eye(128, dtype=bf16),
    ], axis=1))
    in_maps = []
    for c in range(N_CORES):
        b, j = divmod(c, 2)
        sl = q_bf[b, j * SL:(j + 1) * SL, :]
        # xqS: per-pair 128-col slices with a ones column (fused z matmul).
        xqs = np.ones((SL, NPAIR, 129), dtype=bf16)
        xqs[:, :, 0:128] = sl.reshape(SL, NPAIR, 128)
        in_maps.append({
            "xqT": np.ascontiguousarray(sl.T),
            "xqS": np.ascontiguousarray(xqs.reshape(SL, NPAIR * 129)),
            "wpk": wpk,
            "wo": wo_bf,
        })
    return in_maps


def kernel(query, Wq, Wk, Wv, Wo):
    from concourse.bass_utils import run_bass_kernel_spmd

    nc = _get_program()
    in_maps = _host_prep(query, Wq, Wk, Wv, Wo)
    res = run_bass_kernel_spmd(nc, in_maps, list(range(N_CORES)))
    out = np.empty((B, S, E), dtype=np.float32)
    for c in range(N_CORES):
        b, j = divmod(c, 2)
        out[b, j * SL:(j + 1) * SL, :] = res.results[c]["y"].astype(np.float32)
    return out


# revision 35
# speedup vs baseline: 1.1226x; 1.1226x over previous
"""Multi-head linear attention (elu+1 feature map) on 8 TRN2 NeuronCores.

Sharding: core c handles batch b = c//2, sequence half j = c%2 (2048 rows).
Each core computes q/k projections + phi + partial kv/z for its rows,
AllReduces kv/z across the (b, j) pair, then computes num/den/ctx and the
output projection for its rows. All matmuls in bf16 (fp32 PSUM accumulate).

v2 design notes (vs baseline):
  - v projection/eviction eliminated: kv = phi(k)^T (x Wv) = (phi(k)^T x) Wv.
    M = kf^T [x | 1] accumulates in PSUM per pair (z rides along as col 128);
    M is evicted, transposed on PE (host-provided identity), and one matmul
    with block-diag Wv yields kv.
  - phi(x)=elu(x)+1 computed in 3 passes (E=exp(x) fused with PSUM eviction
    on ACT; u=min(E-1,0) on Pool; r=max(x_psum,u) on DVE; +1 on DVE 4x) or
    4 passes (A=x+1 on ACT; m=min(A,1)-1 on Pool; exp on ACT; max on DVE 2x),
    per-slot choice balancing ACT vs DVE occupancy.
  - k-side is group-outer (chunks stream in with the xsm DMA) with a lag-2
    software pipeline between projection matmuls and M/z matmuls; q-side
    512-col slots are interleaved 1:1 between k-side slots to keep PE fed.
  - num -> out-proj pipelined in 1024-row blocks; y stored as bf16 on two
    DMA queues; host casts to f32.
"""
import numpy as np
import ml_dtypes

B, S, H, Dh = 4, 4096, 16, 64
E = H * Dh
N_CORES = 8
SL = S // 2          # sequence rows per core
NPAIR = H // 2       # head pairs
EPS = 1e-6

NCHUNK = SL // 128   # 128-row chunks (16)
NG = 4               # chunk groups (4 chunks each)

_CACHE = {}


def _build_program():
    import concourse.bacc as bacc
    import concourse.mybir as mybir
    import concourse.tile as tile

    bf16 = mybir.dt.bfloat16
    f32 = mybir.dt.float32
    Act = mybir.ActivationFunctionType
    Alu = mybir.AluOpType

    nc = bacc.Bacc(None, target_bir_lowering=False, num_devices=N_CORES)

    xq = nc.dram_tensor("xqT", [E, SL], bf16, kind="ExternalInput")
    xs = nc.dram_tensor("xqS", [SL, NPAIR * 129], bf16, kind="ExternalInput")
    # wpk: [wkv_bd (8x256) | wq_bd (8x128) | ident (128)] packed per partition
    wpk = nc.dram_tensor("wpk", [128, 3200], bf16, kind="ExternalInput")
    wo = nc.dram_tensor("wo", [E, E], bf16, kind="ExternalInput")
    y = nc.dram_tensor("y", [SL, E], bf16, kind="ExternalOutput")
    kv_ar = nc.dram_tensor("kv_ar", [128, NPAIR * 129], bf16)

    with tile.TileContext(nc) as tc:
        with (
            tc.tile_pool(name="persist", bufs=1) as persist,
            tc.tile_pool(name="xp", bufs=1) as xp,
            tc.tile_pool(name="kfp", bufs=3) as kfp,
            tc.tile_pool(name="tmp", bufs=2) as tmp,
            tc.tile_pool(name="qtmp", bufs=2) as qtmp,
            tc.tile_pool(name="mtl", bufs=1) as mtl,
            tc.tile_pool(name="rbcp", bufs=2) as rbcp,
            tc.tile_pool(name="outp", bufs=3) as outp,
            tc.tile_pool(name="dram", bufs=1, space="DRAM") as dram,
        ):
            # ---- input loads, ordered for startup latency ----
            # sync queue: packed weights, then xT in two-pair blocks.
            wpk_sb = persist.tile([128, 3200], bf16)
            nc.sync.dma_start(out=wpk_sb[:], in_=wpk[:, :])
            wkv_sb = wpk_sb[:, 0:2048].rearrange("k (p m) -> k p m", m=256)
            wq_sb = wpk_sb[:, 2048:3072].rearrange("k (p m) -> k p m", m=128)
            ident_sb = wpk_sb[:, 3072:3200]
            # sync carries the critical path alone (wpk + xT pairs 0-1) so
            # the first k-slot starts ~7us; xsm (first needed at slot 2)
            # follows on sync; xT pairs 2-7 ride the scalar queue.
            xq_r = xq.rearrange("(p k) s -> k p s", k=128)
            xsm = persist.tile([128, NCHUNK, NPAIR, 129], bf16)
            xs_r = xs.rearrange("(c s) (p m) -> s c p m", s=128, m=129)
            xTs = []
            for h in range(4):
                xTh = xp.tile([128, 2, SL], bf16, tag=f"xTh{h}")
                eng = nc.sync if h == 0 else nc.scalar
                eng.dma_start(out=xTh[:], in_=xq_r[:, 2 * h:2 * h + 2])
                xTs.extend([xTh[:, 0, :], xTh[:, 1, :]])
            for g in range(NG):
                nc.sync.dma_start(
                    out=xsm[:, 4 * g:4 * (g + 1)], in_=xs_r[:, 4 * g:4 * (g + 1)]
                )
            # sync queue tail: Wo (needed only at out-proj).
            wo_sb = persist.tile([128, NPAIR, E], bf16)
            nc.sync.dma_start(
                out=wo_sb[:], in_=wo.rearrange("(k p) n -> p k n", p=128)
            )
            qfT = persist.tile([128, NPAIR, SL], bf16)
            ctxT = persist.tile([128, NPAIR, SL], bf16)
            kv_in = dram.tile([128, NPAIR * 129], bf16)

            # ---- phase K: k-side slots (two 4-pair waves), then phase Q ----
            # k-slot (g,p): 4 proj matmuls -> phi -> (lag-2) M+z matmuls.
            # Collective for each wave issues right after its tails and hides
            # under the remaining k-slots / q-phase.
            kvrd = persist.tile([128, NPAIR, 129], bf16)
            zbd = persist.tile([128, NPAIR, H], bf16)
            kvbd = persist.tile([128, NPAIR, 128], bf16)
            nc.vector.memset(zbd[:], 0.0)
            nc.vector.memset(kvbd[:], 0.0)
            groups = [[0, 1], [2, 3], [4, 5], [6, 7]]
            with (
                tc.tile_pool(name="ps_kf", bufs=2, space="PSUM") as ps_kf,
                tc.tile_pool(name="ps_m", bufs=1, space="PSUM") as ps_m,
            ):
                def k_proj(g, p, mode4, a_dve=False):
                    kfps = ps_kf.tile([128, 1024], f32, tag="kfps")
                    for c8 in range(8):
                        c = g * 8 + c8
                        nc.tensor.matmul(
                            kfps[:, c8 * 128:(c8 + 1) * 128],
                            lhsT=xTs[p][:, c * 128:(c + 1) * 128],
                            rhs=wkv_sb[:, p, 0:128],
                            start=True, stop=True,
                        )
                    kf = kfp.tile([128, 8, 128], bf16, tag="kf")
                    if mode4:
                        # A=x+1 (ACT fast PSUM path or DVE, alternating to
                        # balance engines); m=min(A,1)-1 (DVE 4x);
                        # exp in-place (ACT 1x); kf=max(A,m) (DVE 2x)
                        kA = tmp.tile([128, 1024], bf16, tag="Ek")
                        if a_dve:
                            nc.vector.tensor_scalar(
                                kA[:], kfps[:], 1.0, None, Alu.add
                            )
                        else:
                            nc.scalar.activation(
                                kA[:], kfps[:], Act.Identity, bias=1.0
                            )
                        kM = tmp.tile([128, 1024], bf16, tag="uk")
                        nc.vector.tensor_scalar(
                            kM[:], kA[:], 1.0, -1.0, Alu.min, Alu.add
                        )
                        nc.scalar.activation(kM[:], kM[:], Act.Exp)
                        nc.vector.tensor_tensor(kf[:], kA[:], kM[:], Alu.max)
                    else:
                        # E=exp(x) (ACT, evicts PSUM); u=min(E-1,0) (DVE 4x);
                        # r=max(x,u) (DVE PSUM); kf=r+1 (DVE 4x)
                        Ek = tmp.tile([128, 1024], bf16, tag="Ek")
                        nc.scalar.activation(Ek[:], kfps[:], Act.Exp)
                        uk = tmp.tile([128, 1024], bf16, tag="uk")
                        nc.vector.tensor_scalar(
                            uk[:], Ek[:], -1.0, 0.0, Alu.add, Alu.min
                        )
                        nc.vector.tensor_tensor(kf[:], kfps[:], uk[:], Alu.max)
                        nc.vector.tensor_scalar(kf[:], kf[:], 1.0, None, Alu.add)
                    return kf

                def m_mm(g, p, kf, M_t):
                    # fused M+z accumulation: rhs = [x-slice | ones] (129 cols)
                    for c8 in range(8):
                        c = g * 8 + c8
                        nc.tensor.matmul(
                            M_t[:, 0:129],
                            lhsT=kf[:, c8, :],
                            rhs=xsm[:, c, p, :],
                            start=(c == 0), stop=(c == NCHUNK - 1),
                        )

                def tails(wave, M_ts):
                    # stage-batched across the 4 pairs: each stage runs on one
                    # engine across the 4 bank-exclusive M tiles in parallel
                    M_sbs, Mt_sbs = [], []
                    for pi in range(4):
                        M_sb = mtl.tile([128, 129], bf16, tag=f"Msb{pi}",
                                        name=f"Msb{pi}")
                        nc.vector.tensor_copy(M_sb[:], M_ts[pi][:, 0:129])
                        M_sbs.append(M_sb)
                    for pi in range(4):
                        Mt_ps = M_ts[pi][:, 192:256].bitcast(bf16)
                        nc.tensor.transpose(Mt_ps, M_sbs[pi][:, 0:128], ident_sb)
                    for pi in range(4):
                        Mt_sb = mtl.tile([128, 128], bf16, tag=f"Mtsb{pi}",
                                         name=f"Mtsb{pi}")
                        nc.vector.tensor_copy(
                            Mt_sb[:], M_ts[pi][:, 192:256].bitcast(bf16)
                        )
                        Mt_sbs.append(Mt_sb)
                    for pi in range(4):
                        p = wave * 4 + pi
                        kvps = M_ts[pi][:, 256:384]
                        nc.tensor.matmul(
                            kvps, lhsT=Mt_sbs[pi][:], rhs=wkv_sb[:, p, 128:256],
                            start=True, stop=True,
                        )
                        kvst = outp.tile([128, 129], bf16, tag="kvst")
                        nc.vector.tensor_copy(kvst[:, 0:128], kvps)
                        nc.vector.tensor_copy(
                            kvst[:, 128:129], M_sbs[pi][:, 128:129]
                        )
                        nc.scalar.dma_start(
                            out=kv_in[:, p * 129:(p + 1) * 129], in_=kvst[:]
                        )

                slot = 0
                for wave in range(2):
                    M_ts = [
                        ps_m.tile([128, 512], f32, tag=f"M{i}", name=f"M{i}")
                        for i in range(4)
                    ]
                    pend = []
                    for g in range(2):
                        for pi in range(4):
                            p = wave * 4 + pi
                            kf = k_proj(g, p, mode4=(slot % 8 < 5), a_dve=(slot % 2 == 1))
                            slot += 1
                            pend.append((g, p, kf))
                            if len(pend) > 2:
                                gg, pp, kk = pend.pop(0)
                                m_mm(gg, pp, kk, M_ts[pp % 4])
                    for gg, pp, kk in pend:
                        m_mm(gg, pp, kk, M_ts[pp % 4])
                    tails(wave, M_ts)
                # AllReduce kv/z; latency hides under the q phase.
                nc.gpsimd.collective_compute(
                    "AllReduce", Alu.add, replica_groups=groups,
                    ins=[kv_in[:]], outs=[kv_ar[:]],
                )
                nc.scalar.dma_start(
                    out=kvrd[:], in_=kv_ar.rearrange("q (p c) -> q p c", c=129)
                )

            # ---- phase Q: qf slots; den accumulates per 4-pair wave so the
            # collective latency hides under the q elementwise work ----
            with (
                tc.tile_pool(name="ps_q", bufs=2, space="PSUM") as ps_q,
                tc.tile_pool(name="ps_den", bufs=1, space="PSUM") as ps_den,
            ):
                denps = ps_den.tile([16, SL], f32)

                def q_slot(j, mode4):
                    p, qq = j // 2, j % 2
                    qs = slice(qq * 1024, (qq + 1) * 1024)
                    qps = ps_q.tile([128, 1024], f32, tag="qps")
                    nc.tensor.matmul(
                        qps[:, 0:512], lhsT=wq_sb[:, p, :],
                        rhs=xTs[p][:, qq * 1024:qq * 1024 + 512],
                        start=True, stop=True,
                    )
                    nc.tensor.matmul(
                        qps[:, 512:1024], lhsT=wq_sb[:, p, :],
                        rhs=xTs[p][:, qq * 1024 + 512:(qq + 1) * 1024],
                        start=True, stop=True,
                    )
                    if mode4:
                        qA = qtmp.tile([128, 1024], bf16, tag="qA")
                        if j % 2 == 1:
                            nc.vector.tensor_scalar(
                                qA[:], qps[:], 1.0, None, Alu.add
                            )
                        else:
                            nc.scalar.activation(
                                qA[:], qps[:], Act.Identity, bias=1.0
                            )
                        qM = qtmp.tile([128, 1024], bf16, tag="qM")
                        nc.vector.tensor_scalar(
                            qM[:], qA[:], 1.0, -1.0, Alu.min, Alu.add
                        )
                        nc.scalar.activation(qM[:], qM[:], Act.Exp)
                        nc.vector.tensor_tensor(
                            qfT[:, p, qs], qA[:], qM[:], Alu.max
                        )
                    else:
                        Eq = qtmp.tile([128, 1024], bf16, tag="qA")
                        nc.scalar.activation(Eq[:], qps[:], Act.Exp)
                        uq = qtmp.tile([128, 1024], bf16, tag="qM")
                        nc.vector.tensor_scalar(
                            uq[:], Eq[:], -1.0, 0.0, Alu.add, Alu.min
                        )
                        nc.vector.tensor_tensor(
                            qfT[:, p, qs], qps[:], uq[:], Alu.max
                        )
                        nc.vector.tensor_scalar(
                            qfT[:, p, qs], qfT[:, p, qs], 1.0, None, Alu.add
                        )

                for j in range(16):
                    q_slot(j, mode4=True)
                for p in range(NPAIR):
                    nc.vector.tensor_copy(
                        zbd[0:64, p, 2 * p:2 * p + 1], kvrd[0:64, p, 128:129]
                    )
                    nc.vector.tensor_copy(
                        zbd[64:128, p, 2 * p + 1:2 * p + 2],
                        kvrd[64:128, p, 128:129],
                    )
                    nc.vector.tensor_copy(
                        kvbd[0:64, p, 0:64], kvrd[0:64, p, 0:64]
                    )
                    nc.vector.tensor_copy(
                        kvbd[64:128, p, 64:128], kvrd[64:128, p, 64:128]
                    )
                # den/recip/store pipelined per 1024-column half so the
                # first num block starts as early as possible
                den_sb = persist.tile([16, SL], f32)
                recip = persist.tile([16, SL], bf16)
                eps_sb = persist.tile([16, 1], f32)
                nc.vector.memset(eps_sb[:], EPS)
                recip_dram = dram.tile([16, SL], bf16)
                for qch in range(2):
                    hs = slice(qch * 1024, (qch + 1) * 1024)
                    for p in range(NPAIR):
                        for q2 in range(2):
                            qs = slice(
                                qch * 1024 + q2 * 512, qch * 1024 + (q2 + 1) * 512
                            )
                            nc.tensor.matmul(
                                denps[:, qs], lhsT=zbd[:, p, :],
                                rhs=qfT[:, p, qs],
                                start=(p == 0), stop=(p == NPAIR - 1),
                            )
                    nc.scalar.activation(
                        den_sb[:, hs], denps[:, hs], Act.Identity, bias=eps_sb[:]
                    )
                    eng = nc.scalar
                    eng.add_instruction(
                        mybir.InstActivation(
                            name=nc.get_next_instruction_name(),
                            func=Act.Reciprocal,
                            ins=[
                                eng.lower_ap(den_sb[:, hs]),
                                mybir.ImmediateValue(dtype=f32, value=0.0),
                                mybir.ImmediateValue(dtype=f32, value=1.0),
                                mybir.ImmediateValue(dtype=f32, value=0.0),
                            ],
                            outs=[eng.lower_ap(recip[:, hs])],
                        )
                    )
                    nc.sync.dma_start(
                        out=recip_dram[:, hs], in_=recip[:, hs]
                    )

            # ---- num + ctx + output projection, pipelined per 1024 rows ----
            with (
                tc.tile_pool(name="ps_num", bufs=2, space="PSUM") as ps_num,
                tc.tile_pool(name="ps_o", bufs=2, space="PSUM") as ps_o,
            ):
                for qc in range(2):
                    qs = slice(qc * 1024, (qc + 1) * 1024)
                    for p in range(NPAIR):
                        rbc = rbcp.tile([128, 1024], bf16, tag="rbc")
                        eng = nc.sync if p % 2 == 0 else nc.scalar
                        eng.dma_start(
                            out=rbc[0:64, :],
                            in_=recip_dram[2 * p:2 * p + 1, qs].to_broadcast(
                                [64, 1024]
                            ),
                        )
                        eng.dma_start(
                            out=rbc[64:128, :],
                            in_=recip_dram[2 * p + 1:2 * p + 2, qs].to_broadcast(
                                [64, 1024]
                            ),
                        )
                        nps = ps_num.tile([128, 1024], f32, tag="nps")
                        nc.tensor.matmul(
                            nps[:, 0:512], lhsT=kvbd[:, p, :],
                            rhs=qfT[:, p, qc * 1024:qc * 1024 + 512],
                            start=True, stop=True,
                        )
                        nc.tensor.matmul(
                            nps[:, 512:1024], lhsT=kvbd[:, p, :],
                            rhs=qfT[:, p, qc * 1024 + 512:(qc + 1) * 1024],
                            start=True, stop=True,
                        )
                        nc.vector.tensor_tensor(
                            ctxT[:, p, qs], nps[:], rbc[:], Alu.mult
                        )
                    for s8 in range(8):
                        si = qc * 8 + s8
                        ss = slice(si * 128, (si + 1) * 128)
                        ops = ps_o.tile([128, E], f32, tag="ops")
                        for k in range(NPAIR):
                            nc.tensor.matmul(
                                ops[:, 0:512], lhsT=ctxT[:, k, ss],
                                rhs=wo_sb[:, k, 0:512],
                                start=(k == 0), stop=(k == NPAIR - 1),
                            )
                            nc.tensor.matmul(
                                ops[:, 512:E], lhsT=ctxT[:, k, ss],
                                rhs=wo_sb[:, k, 512:E],
                                start=(k == 0), stop=(k == NPAIR - 1),
                            )
                        ysb = outp.tile([128, E], bf16, tag="ysb")
                        nc.scalar.activation(ysb[:], ops[:], Act.Identity)
                        eng = nc.sync if s8 % 2 == 0 else nc.scalar
                        eng.dma_start(out=y[ss, :], in_=ysb[:])

    nc.compile()
    return nc


def _get_program():
    if "nc" not in _CACHE:
        _CACHE["nc"] = _build_program()
    return _CACHE["nc"]


def _host_prep(query, Wq, Wk, Wv, Wo):
    bf16 = ml_dtypes.bfloat16
    q_bf = np.ascontiguousarray(np.asarray(query).astype(bf16))
    wq_bd = np.zeros((NPAIR, 128, 128), dtype=bf16)
    wkv_bd = np.zeros((NPAIR, 128, 256), dtype=bf16)
    for p in range(NPAIR):
        wq_bd[p, 0:64, 0:64] = Wq[2 * p]
        wq_bd[p, 64:128, 64:128] = Wq[2 * p + 1]
        wkv_bd[p, 0:64, 0:64] = Wk[2 * p]
        wkv_bd[p, 64:128, 64:128] = Wk[2 * p + 1]
        wkv_bd[p, 0:64, 128:192] = Wv[2 * p]
        wkv_bd[p, 64:128, 192:256] = Wv[2 * p + 1]
    wo_bf = np.ascontiguousarray(np.asarray(Wo).astype(bf16))
    wpk = np.ascontiguousarray(np.concatenate([
        wkv_bd.transpose(1, 0, 2).reshape(128, NPAIR * 256),
        wq_bd.transpose(1, 0, 2).reshape(128, NPAIR * 128),
        np.eye(128, dtype=bf16),
    ], axis=1))
    in_maps = []
    for c in range(N_CORES):
        b, j = divmod(c, 2)
        sl = q_bf[b, j * SL:(j + 1) * SL, :]
        # xqS: per-pair 128-col slices with a ones column (fused z matmul).
        xqs = np.ones((SL, NPAIR, 129), dtype=bf16)
        xqs[:, :, 0:128] = sl.reshape(SL, NPAIR, 128)
        in_maps.append({
            "xqT": np.ascontiguousarray(sl.T),
            "xqS": np.ascontiguousarray(xqs.reshape(SL, NPAIR * 129)),
            "wpk": wpk,
            "wo": wo_bf,
        })
    return in_maps


def kernel(query, Wq, Wk, Wv, Wo):
    from concourse.bass_utils import run_bass_kernel_spmd

    nc = _get_program()
    in_maps = _host_prep(query, Wq, Wk, Wv, Wo)
    res = run_bass_kernel_spmd(nc, in_maps, list(range(N_CORES)))
    out = np.empty((B, S, E), dtype=np.float32)
    for c in range(N_CORES):
        b, j = divmod(c, 2)
        out[b, j * SL:(j + 1) * SL, :] = res.results[c]["y"].astype(np.float32)
    return out
